# revision 21
# baseline (speedup 1.0000x reference)
"""Trainium2 Bass kernel for nn_AdaptiveFireRateCAModel (adaptive-fire-rate neural CA).

Data-parallel over batch: 8 images -> 8 NeuronCores, one image per core.
Per-core layout: channel-major padded maps [C, 5480] fp32; pixel (h,w) at
flat col 2 + (h+1)*74 + (w+1)   (74x74 zero pad ring + 2 guard cols).
Conv taps are free-dim offsets dy*74+dx; matmuls use fp32r (11-bit mantissa
operand rounding, full PE rate) with dx folded into M as 3 output variants
combined on DVE. Bernoulli draws replicated exactly via a numpy threefry port.
"""
import numpy as np

# ---------------------------------------------------------------- constants
B, H, W, C = 8, 72, 72, 16
HID_E, HID_P = 128, 16
T = 64
N_STEPS = T + 1          # step 0 (initial update) + 64 recorded steps
P_EPS = 1e-6
ALIVE_TH = 0.1
ROW = 74                 # padded row width
MAPW = 5480              # 2 guard + 74*74 + 2 guard
NCHUNK = 12              # 12 chunks x 6 rows
CH_ROWS = 6
CLEN = CH_ROWS * ROW     # 444
INT_FLAT = H * W         # 5184

_DX = np.outer([1.0, 2.0, 1.0], [-1.0, 0.0, 1.0]).astype(np.float64) / 8.0
W1_EFF = _DX.T.astype(np.float32)   # sobel kernel 1 (applied depthwise)
W2_EFF = _DX.astype(np.float32)     # sobel kernel 2


# ---------------------------------------------------------------- threefry (bitwise jax-compatible)
def _rotl(x, d):
    return ((x << np.uint32(d)) | (x >> np.uint32(32 - d))).astype(np.uint32)


def _threefry2x32(ks0, ks1, x0, x1):
    ks0 = np.uint32(ks0); ks1 = np.uint32(ks1)
    ks2 = np.uint32(ks0 ^ ks1 ^ np.uint32(0x1BD11BDA))
    x0 = (x0 + ks0).astype(np.uint32); x1 = (x1 + ks1).astype(np.uint32)
    rot1 = [13, 15, 26, 6]; rot2 = [17, 29, 16, 24]
    ks = [ks1, ks2, ks0, ks1, ks2, ks0]
    for r in range(5):
        for rot in (rot1 if r % 2 == 0 else rot2):
            x0 = (x0 + x1).astype(np.uint32)
            x1 = _rotl(x1, rot)
            x1 = (x1 ^ x0).astype(np.uint32)
        x0 = (x0 + ks[r]).astype(np.uint32)
        x1 = (x1 + ks[r + 1] + np.uint32(r + 1)).astype(np.uint32)
    return x0, x1


def _fold_in(key, data):
    k0 = np.uint32(np.uint64(data) >> np.uint64(32))
    k1 = np.uint32(np.uint64(data) & np.uint64(0xFFFFFFFF))
    a, b = _threefry2x32(key[0], key[1], np.array([k0], np.uint32), np.array([k1], np.uint32))
    return np.uint32(a[0]), np.uint32(b[0])


def _uniform01(key, n):
    idx = np.arange(n, dtype=np.uint64)
    x0 = (idx >> np.uint64(32)).astype(np.uint32)
    x1 = (idx & np.uint64(0xFFFFFFFF)).astype(np.uint32)
    a, b = _threefry2x32(key[0], key[1], x0, x1)
    bits = a ^ b
    return (((bits >> np.uint32(9)) | np.uint32(0x3F800000)).view(np.float32)
            - np.float32(1.0))


def _all_uniforms():
    """u_n for n=0..T, shape (N_STEPS, B, H, W) fp32 — bitwise equal to
    jax.random.uniform(fold_in(key(42), n), (B,H,W))."""
    base = (np.uint32(0), np.uint32(42))
    out = np.empty((N_STEPS, B, H, W), np.float32)
    for n in range(N_STEPS):
        out[n] = _uniform01(_fold_in(base, n), B * H * W).reshape(B, H, W)
    return out


# ---------------------------------------------------------------- host-side packing
def _pad_map(img_chw):
    """[C,72,72] -> [C, MAPW] padded flat."""
    c = img_chw.shape[0]
    m = np.zeros((c, MAPW), np.float32)
    v = np.zeros((c, 74, 74), np.float32)
    v[:, 1:73, 1:73] = img_chw
    m[:, 2:2 + 74 * 74] = v.reshape(c, -1)
    return m


def _prep_weights(conv1_w, conv1_b, conv2_w, conv2_b, fc0_w, fc0_b, fc1_w):
    # dx-variant folded weight mats, lhsT layout [K, M]
    # perc channel order on device: [s1(0:16), s2(16:32), x(32:48)]
    perm = np.concatenate([np.arange(16, 32), np.arange(32, 48), np.arange(0, 16)])
    w1v = np.zeros((3, 48, 96), np.float32)   # [dy][perc_c, 32*vdx + o]  (slabs 32-spaced)
    w2v = np.zeros((3, 16, 96), np.float32)   # [dy][h1_c, 32*vdx + replica]
    wsv = np.zeros((3, 16, 96), np.float32)   # [dy][x_c, 32*vdx + (s1:c | s2:16+c)]
    for dy in range(3):
        for v in range(3):
            w1v[dy, :, 32 * v:32 * v + 16] = conv1_w[dy, v][perm]    # [48,16]
            w2v[dy, :, 32 * v:32 * v + 16] = np.repeat(conv2_w[dy, v], 16, axis=1)
            for cch in range(16):
                wsv[dy, cch, 32 * v + cch] = W1_EFF[dy, v]
                wsv[dy, cch, 32 * v + 16 + cch] = W2_EFF[dy, v]
    return dict(
        w1v=w1v, w2v=w2v, wsv=wsv,
        fc0t=fc0_w.astype(np.float32)[perm],      # [48,128]
        fc1t=fc1_w.astype(np.float32),            # [128,16]
        b0=fc0_b.reshape(128, 1).astype(np.float32),
        b1=conv1_b.reshape(16, 1).astype(np.float32),
        b2=np.full((16, 1), np.float32(conv2_b.reshape(-1)[0])),
        ones64=np.ones((1, 64), np.float32),
    )


# ---------------------------------------------------------------- device graph
def _build_nc(n_steps):
    from contextlib import ExitStack
    import concourse.bass as bass
    import concourse.tile as tile
    from concourse import bacc, mybir

    DT = mybir.dt.float32
    DTR = mybir.dt.float32r
    AF = mybir.ActivationFunctionType
    OP = mybir.AluOpType
    NREC = n_steps - 1

    nc = bacc.Bacc("TRN2", target_bir_lowering=False, debug=False)

    xin = nc.declare_dram_parameter("xin", [16, MAPW], DT, isOutput=False)
    uin = nc.declare_dram_parameter("u", [n_steps, 16, INT_FLAT], DT, isOutput=False)
    wpar = {}
    for name, shape in [("w1v", [3, 48, 96]), ("w2v", [3, 16, 96]), ("wsv", [3, 16, 96]),
                        ("fc0t", [48, 128]), ("fc1t", [128, 16]),
                        ("b0", [128, 1]), ("b1", [16, 1]), ("b2", [16, 1]),
                        ("ones64", [1, 64])]:
        wpar[name] = nc.declare_dram_parameter(name, shape, DT, isOutput=False)

    xs_out = nc.declare_dram_parameter("xs", [NREC, 16, H, W], DT, isOutput=True)
    lam_out = nc.declare_dram_parameter("lam", [NREC, H, W], DT, isOutput=True)
    upd_out = nc.declare_dram_parameter("upd", [NREC, H, W], DT, isOutput=True)
    pn_out = nc.declare_dram_parameter("pn", [NREC, H, W], DT, isOutput=True)
    t_dram = nc.dram_tensor("t_bounce", [NREC, INT_FLAT], DT)

    with tile.TileContext(nc) as tc:
        with ExitStack() as ctx:
            wp = ctx.enter_context(tc.tile_pool(name="weights", bufs=1))
            st = ctx.enter_context(tc.tile_pool(name="state", bufs=1))
            sm = ctx.enter_context(tc.tile_pool(name="small", bufs=1))
            utp = ctx.enter_context(tc.tile_pool(name="upool", bufs=2))
            hp = ctx.enter_context(tc.tile_pool(name="hpool", bufs=2))
            tmp = ctx.enter_context(tc.tile_pool(name="tmp", bufs=3))
            pconv = ctx.enter_context(tc.tile_pool(name="pconv", bufs=3, space="PSUM"))
            pmlp = ctx.enter_context(tc.tile_pool(name="pmlp", bufs=2, space="PSUM"))
            pdx = ctx.enter_context(tc.tile_pool(name="pdx", bufs=2, space="PSUM"))

            # ---- persistent weights in SBUF (cast to fp32r via gpsimd DMA)
            w1v = [wp.tile([48, 96], DTR, tag=f"w1v{d}", name=f"w1v{d}") for d in range(3)]
            w2v = [wp.tile([16, 96], DTR, tag=f"w2v{d}", name=f"w2v{d}") for d in range(3)]
            wsv = [wp.tile([48, 96], DTR, tag=f"wsv{d}", name=f"wsv{d}") for d in range(3)]
            for d in range(3):
                nc.gpsimd.dma_start(w1v[d][:], wpar["w1v"][d])
                nc.gpsimd.dma_start(w2v[d][:], wpar["w2v"][d])
                nc.gpsimd.dma_start(wsv[d][32:48, :], wpar["wsv"][d])
            fc0t = wp.tile([48, 128], DTR, tag="fc0t", name="fc0t")
            fc1t = wp.tile([128, 16], DTR, tag="fc1t", name="fc1t")
            ones64 = wp.tile([1, 64], DTR, tag="ones64", name="ones64")
            ones64f = wp.tile([1, 64], DT, tag="ones64f", name="ones64f")
            nc.gpsimd.dma_start(fc0t[:], wpar["fc0t"][:])
            nc.gpsimd.dma_start(fc1t[:], wpar["fc1t"][:])
            nc.gpsimd.dma_start(ones64[:], wpar["ones64"][:])
            nc.sync.dma_start(ones64f[:], wpar["ones64"][:])
            b0 = wp.tile([128, 1], DT, tag="b0", name="b0")
            b1 = wp.tile([16, 1], DT, tag="b1", name="b1")
            b2 = wp.tile([16, 1], DT, tag="b2", name="b2")
            nc.sync.dma_start(b0[:], wpar["b0"][:])
            nc.sync.dma_start(b1[:], wpar["b1"][:])
            nc.sync.dma_start(b2[:], wpar["b2"][:])

            # ---- persistent state
            perc = [st.tile([48, MAPW], DTR, tag="percA", name="percA"),
                    st.tile([48, MAPW], DTR, tag="percB", name="percB")]
            xf = st.tile([16, MAPW], DT, tag="xf", name="xf")       # exact fp32 x
            h1 = st.tile([16, MAPW], DTR, tag="h1", name="h1")
            xn16 = st.tile([16, MAPW], DT, tag="xn16", name="xn16")
            lam16 = st.tile([16, MAPW], DT, tag="lam16", name="lam16")
            upd16 = st.tile([16, MAPW], DT, tag="upd16", name="upd16")
            un1 = st.tile([1, INT_FLAT], DT, tag="un1", name="un1")   # interior-flat un

            a72 = [sm.tile([72, 74], DT, tag=f"a72_{i}", name=f"a72_{i}") for i in range(2)]
            mp = [sm.tile([72, 74], DT, tag=f"mp{i}", name=f"mp{i}") for i in range(4)]
            pre72 = sm.tile([72, 74], DT, tag="pre72", name="pre72")
            lif72 = sm.tile([72, 74], DT, tag="lif72", name="lif72")

            for t_ in (perc[0], perc[1], xf, h1, xn16, lam16, upd16,
                       a72[0], a72[1], mp[0], mp[1], mp[2], mp[3], pre72, lif72):
                ap = t_[:]
                if ap.dtype == DTR:
                    ap = ap.bitcast(DT)
                nc.gpsimd.memset(ap, 0.0)
            nc.gpsimd.memset(un1[:], 1.0)

            # 3-D views (74 rows x 74 cols)
            def v3(t, p0, p1):
                return t[p0:p1, 2:2 + 74 * 74].rearrange("p (h w) -> p h w", h=74)

            xf3 = v3(xf, 0, 16)
            xn3 = v3(xn16, 0, 16)
            lam3 = v3(lam16, 0, 16)
            upd3 = v3(upd16, 0, 16)
            unF3 = un1[0:1, :].rearrange("p (h w) -> p h w", h=72)
            # t = -un*lam scratch lives in upd16 row 0 (holds prev-step upd,
            # already DMA'd out by the time t is written)
            tF3 = v3(upd16, 0, 1)

            # load x into xf (fp32) and perc[1] x-block (fp32r)
            nc.sync.dma_start(xf[:], xin[:])
            nc.gpsimd.dma_start(perc[1][32:48, :], xin[:])

            def chunk_cols(j):
                return 2 + ROW + CLEN * j          # start col of chunk j (6 rows incl pads)

            def conv3(dst_psum, m, src, src_p0, src_k, wts, j, extend=1):
                """3 dy-matmuls accumulating into psum [m, CLEN+2*extend]."""
                c0 = chunk_cols(j) - extend
                ln = CLEN + 2 * extend
                for di, dy in enumerate((-1, 0, 1)):
                    s = c0 + dy * ROW
                    nc.tensor.matmul(dst_psum[0:m, 0:ln],
                                     wts[di][src_p0:src_p0 + src_k, :],
                                     src[src_p0:src_p0 + src_k, s:s + ln],
                                     start=(di == 0), stop=(di == 2))

            def combine3(psum, nslab, dst3, dst_p0, rows_j, bias_ap, act, dtag):
                """dst3[:, r0:r0+6, 1:73] = act(V-1[c-1]+V0[c]+V1[c+1]+bias).
                One PSUM operand per instruction: evacuate slab0 via ACT."""
                w_ = nslab
                tA = tmp.tile([w_, CLEN + 2], DT, tag="tmp", name=f"cmbA{dtag}")
                if bias_ap is not None:
                    nc.scalar.activation(tA[:, 0:CLEN], psum[0:w_, 0:CLEN],
                                         AF.Identity, bias=bias_ap)
                else:
                    nc.scalar.activation(tA[:, 0:CLEN], psum[0:w_, 0:CLEN],
                                         AF.Identity)
                tB = tmp.tile([w_, CLEN + 2], DT, tag="tmp", name=f"cmbB{dtag}")
                nc.vector.tensor_tensor(
                    tB[:, 0:CLEN], tA[:, 0:CLEN],
                    psum[32:32 + w_, 1:1 + CLEN], OP.add)
                r0 = 1 + CH_ROWS * rows_j
                dst = dst3[dst_p0:dst_p0 + w_, r0:r0 + CH_ROWS, 1:73]
                if act is None:
                    tC = tB  # reuse: write final sum straight to dst
                    nc.vector.tensor_tensor(
                        tA[:, 0:CLEN], tB[:, 0:CLEN],
                        psum[64:64 + w_, 2:2 + CLEN], OP.add)
                    tCv = tA[:, 0:CLEN].rearrange("p (h w) -> p h w", h=CH_ROWS)
                    nc.vector.tensor_copy(dst, tCv[:, :, 1:73])
                else:
                    tC = tmp.tile([w_, CLEN + 2], DT, tag="tmp", name=f"cmbC{dtag}")
                    nc.vector.tensor_tensor(
                        tC[:, 0:CLEN], tB[:, 0:CLEN],
                        psum[64:64 + w_, 2:2 + CLEN], OP.add)
                    tCv = tC[:, 0:CLEN].rearrange("p (h w) -> p h w", h=CH_ROWS)
                    nc.scalar.activation(dst, tCv[:, :, 1:73], act)

            def maxpool_gt(src72, out72, w1t, w2t, shu, shd):
                """out72 = (3x3 maxpool(src) > ALIVE_TH); DMA row shifts for the
                h direction (engine partition bases must be 32-aligned)."""
                nc.vector.tensor_tensor(w1t[:, 1:73], src72[:, 0:72], src72[:, 2:74], OP.max)
                nc.vector.tensor_tensor(w2t[:, 1:73], w1t[:, 1:73], src72[:, 1:73], OP.max)
                nc.sync.dma_start(shu[0:71, 1:73], w2t[1:72, 1:73])
                nc.sync.dma_start(shd[1:72, 1:73], w2t[0:71, 1:73])
                nc.vector.tensor_tensor(w1t[:, 1:73], shu[:, 1:73], shd[:, 1:73], OP.max)
                nc.vector.tensor_tensor(w1t[:, 1:73], w1t[:, 1:73], w2t[:, 1:73], OP.max)
                nc.vector.tensor_scalar(out72[:, 1:73], w1t[:, 1:73], ALIVE_TH, None, OP.is_gt)

            # ---------------- one full step ----------------
            def body(n):
                rec = n >= 1
                cur = perc[1] if n <= 1 else (perc[0] if n % 2 == 0 else perc[1])
                nxt = perc[1] if n == 0 else (perc[0] if n % 2 == 1 else perc[1])
                xdst = perc[0] if n % 2 == 0 else perc[1]
                # note: cur=home(X_{n-1}), nxt=home(X_n), xdst=home(X_{n+1})

                # --- record prev-step upd + this step's x BEFORE overwrites
                if rec:
                    nc.sync.dma_start(upd_out[n - 1], upd3[0:1, 1:73, 1:73])
                    nc.sync.dma_start(xs_out[n - 1], xf3[:, 1:73, 1:73])


                # --- pre-alive from current x
                nc.sync.dma_start(a72[0][:, 1:73], xf3[3:4, 1:73, 1:73])
                maxpool_gt(a72[0], pre72, mp[0], mp[1], mp[2], mp[3])

                # --- lambda path: conv1 on stale perc -> h1 -> conv2 -> lam
                for j in range(NCHUNK):
                    pc = pconv.tile([96, CLEN + 2], DT, tag="pconv", name="pconv")
                    conv3(pc, 96, cur, 0, 48, w1v, j)
                    combine3(pc, 16, v3(h1, 0, 16), 0, j, b1[:, 0:1], AF.Relu, "h1")
                for j in range(NCHUNK):
                    pc = pconv.tile([96, CLEN + 2], DT, tag="pconv", name="pconv")
                    conv3(pc, 96, h1, 0, 16, w2v, j)
                    combine3(pc, 16, lam3, 0, j, b2[:, 0:1], AF.Sigmoid, "lam")

                # --- un / t / upd (pointwise)
                if rec:
                    nc.vector.scalar_tensor_tensor(
                        tF3[:, 1:73, 1:73], lam3[0:1, 1:73, 1:73], -1.0,
                        unF3[:, :, :], OP.mult, OP.mult)        # t = -lam*un
                    nc.vector.tensor_tensor(
                        unF3[:, :, :], unF3[:, :, :], tF3[:, 1:73, 1:73], OP.add)  # un += t
                    nc.sync.dma_start(lam_out[n - 1], lam3[0:1, 1:73, 1:73])
                    nc.sync.dma_start(t_dram[n - 1], tF3[0:1, 1:73, 1:73])
                for j in range(NCHUNK):
                    uc = utp.tile([16, 432], DT, tag="uc", name="uc")
                    nc.sync.dma_start(uc[:], uin[n, :, 432 * j:432 * (j + 1)])
                    r0 = 1 + CH_ROWS * j
                    uc3 = uc[:, :].rearrange("p (h w) -> p h w", h=CH_ROWS)
                    nc.vector.tensor_tensor(
                        upd3[:, r0:r0 + CH_ROWS, 1:73], uc3[:, :, :],
                        lam3[:, r0:r0 + CH_ROWS, 1:73], OP.is_ge)

                # --- sobel: perceive X_n s-blocks into nxt
                if n >= 1:
                    for j in range(NCHUNK):
                        ps = pconv.tile([96, CLEN + 2], DT, tag="pconv", name="pconv")
                        conv3(ps, 96, nxt, 32, 16, wsv, j)
                        combine3(ps, 32, v3(nxt, 0, 32), 0, j, None, None, "sob")

                # --- MLP + x update per chunk
                for j in range(NCHUNK):
                    c0 = chunk_cols(j)
                    ph = pmlp.tile([128, CLEN], DT, tag="ph", name="ph")
                    nc.tensor.matmul(ph[:], fc0t[:], nxt[0:48, c0:c0 + CLEN],
                                     start=True, stop=True)
                    ht = hp.tile([128, CLEN], DTR, tag="ht", name="ht")
                    nc.scalar.activation(ht[:], ph[:], AF.Relu, bias=b0[:, 0:1])
                    pd = pdx.tile([16, CLEN], DT, tag="pd", name="pd")
                    nc.tensor.matmul(pd[:], fc1t[:], ht[:], start=True, stop=True)
                    # xn = x + dx*upd   (interior rows of this chunk)
                    r0 = 1 + CH_ROWS * j
                    pd3 = pd[:, :].rearrange("p (h w) -> p h w", h=CH_ROWS)
                    tdx = tmp.tile([16, CLEN], DT, tag="tmp", name="tdx")
                    tdx3 = tdx[:, :].rearrange("p (h w) -> p h w", h=CH_ROWS)
                    nc.vector.tensor_tensor(
                        tdx3[:, :, 1:73], pd3[:, :, 1:73],
                        upd3[:, r0:r0 + CH_ROWS, 1:73], OP.mult)
                    nc.vector.tensor_tensor(
                        xn3[:, r0:r0 + CH_ROWS, 1:73], tdx3[:, :, 1:73],
                        xf3[:, r0:r0 + CH_ROWS, 1:73], OP.add)

                # --- life = pre & alive(xn)
                nc.sync.dma_start(a72[1][:, 1:73], xn3[3:4, 1:73, 1:73])
                maxpool_gt(a72[1], lif72, mp[0], mp[1], mp[2], mp[3])
                nc.vector.tensor_tensor(lif72[:, 1:73], lif72[:, 1:73],
                                        pre72[:, 1:73], OP.mult)
                # broadcast life -> 16 partitions via K=1 matmul, then
                # multiply xn by life straight out of PSUM per chunk
                ones16 = ones64[0:1, 0:16]
                xd3 = v3(xdst, 32, 48)
                for j in range(NCHUNK):
                    lifc = utp.tile([1, 432], DTR, tag="lifc", name="lifc")
                    nc.gpsimd.dma_start(lifc[:], lif72[6 * j:6 * j + 6, 1:73])
                    pb = pdx.tile([16, CLEN], DT, tag="pd", name="pd")
                    nc.tensor.matmul(pb[0:16, 0:432], ones16, lifc[:],
                                     start=True, stop=True)
                    r0 = 1 + CH_ROWS * j
                    pb3 = pb[:, 0:432].rearrange("p (h w) -> p h w", h=CH_ROWS)
                    nc.vector.tensor_tensor(xf3[:, r0:r0 + CH_ROWS, 1:73],
                                            xn3[:, r0:r0 + CH_ROWS, 1:73],
                                            pb3[:, :, :], OP.mult)
                    nc.vector.tensor_tensor(xd3[:, r0:r0 + CH_ROWS, 1:73],
                                            xn3[:, r0:r0 + CH_ROWS, 1:73],
                                            pb3[:, :, :], OP.mult)

            # initial sobel of x_orig into perc[1] (body n=0 skips sobel)
            for j in range(NCHUNK):
                ps = pconv.tile([96, CLEN + 2], DT, tag="pconv", name="pconv")
                conv3(ps, 96, perc[1], 32, 16, wsv, j)
                combine3(ps, 32, v3(perc[1], 0, 32), 0, j, None, None, "sob")

            for n in range(n_steps):
                body(n)

            # ---------------- p normalization ----------------
            # t_n = -q_n;  P_n = (t_n - eps) * recip(sum_n t_n - 64*eps)
            import bass_rust
            ts = st.tile([64, INT_FLAT], DT, tag="lam16", name="ts")
            if NREC < 64:
                nc.gpsimd.memset(ts[:], 0.0)
            nc.sync.dma_start(ts[0:NREC, :], t_dram[:, :])
            sr64 = st.tile([64, INT_FLAT], DT, tag="upd16", name="sr64")
            nc.gpsimd.partition_all_reduce(sr64[:], ts[:], 64,
                                           bass_rust.ReduceOp.add)
            nc.vector.tensor_scalar(sr64[:], sr64[:], -float(NREC) * P_EPS,
                                    None, OP.add)
            r64 = st.tile([64, INT_FLAT], DT, tag="xn16", name="r64")
            nc.vector.reciprocal(r64[:], sr64[:])
            po = st.tile([64, INT_FLAT], DT, tag="xf", name="po")
            nc.vector.scalar_tensor_tensor(po[:], ts[:], P_EPS, r64[:],
                                           OP.subtract, OP.mult)
            nc.sync.dma_start(pn_out[:, :, :].rearrange("n h w -> n (h w)"),
                              po[0:NREC, :])
    nc.compile()
    return nc


_NC_CACHE = {}


def _get_nc(n_steps):
    if n_steps not in _NC_CACHE:
        _NC_CACHE[n_steps] = _build_nc(n_steps)
    return _NC_CACHE[n_steps]


# ---------------------------------------------------------------- entry point
def kernel(x, conv1_w, conv1_b, conv2_w, conv2_b, fc0_w, fc0_b, fc1_w,
           n_steps=N_STEPS, _return_raw=False, _trace=False):
    from concourse.bass_utils import run_bass_kernel_spmd

    x = np.asarray(x, np.float32)
    wd = _prep_weights(np.asarray(conv1_w), np.asarray(conv1_b),
                       np.asarray(conv2_w), np.asarray(conv2_b),
                       np.asarray(fc0_w), np.asarray(fc0_b), np.asarray(fc1_w))
    u_all = _all_uniforms()[:n_steps]          # (n_steps, B, H, W)

    nc = _get_nc(n_steps)
    in_maps = []
    for b in range(B):
        m = dict(wd)
        m["xin"] = _pad_map(x[b].transpose(2, 0, 1))
        uflat = u_all[:n_steps, b].reshape(n_steps, 1, INT_FLAT)
        m["u"] = np.repeat(uflat, 16, axis=1).astype(np.float32)
        in_maps.append(m)

    res = run_bass_kernel_spmd(nc, in_maps, core_ids=list(range(B)), trace=_trace)
    nrec = n_steps - 1
    x_steps = np.zeros((nrec, B, H, W, C), np.float32)
    p_norm = np.zeros((nrec, B, H, W), np.float32)
    lam_steps = np.zeros((nrec, B, H, W), np.float32)
    upd_steps = np.zeros((nrec, B, H, W), np.float32)
    for b in range(B):
        r = res.results[b]
        x_steps[:, b] = r["xs"].transpose(0, 2, 3, 1)
        p_norm[:, b] = r["pn"]
        lam_steps[:, b] = r["lam"]
        upd_steps[:, b] = r["upd"]
    if _return_raw:
        return (x_steps, p_norm, lam_steps, upd_steps), res
    return x_steps, p_norm, lam_steps, upd_steps


# revision 29
# speedup vs baseline: 14.0010x; 14.0010x over previous
"""Trainium2 Bass kernel for nn_AdaptiveFireRateCAModel (adaptive-fire-rate neural CA).

Data-parallel over batch: 8 images -> 8 NeuronCores, one image per core.
Per-core layout: channel-major padded maps [C, 5480] fp32; pixel (h,w) at
flat col 2 + (h+1)*74 + (w+1)   (74x74 zero pad ring + 2 guard cols).
Conv taps are free-dim offsets dy*74+dx; matmuls use fp32r (11-bit mantissa
operand rounding, full PE rate) with dx folded into M as 3 output variants
combined on DVE. Bernoulli draws replicated exactly via a numpy threefry port.
"""
import numpy as np

# ---------------------------------------------------------------- constants
B, H, W, C = 8, 72, 72, 16
HID_E, HID_P = 128, 16
T = 64
N_STEPS = T + 1          # step 0 (initial update) + 64 recorded steps
P_EPS = 1e-6
ALIVE_TH = 0.1
ROW = 74                 # padded row width
MAPW = 5480              # 2 guard + 74*74 + 2 guard
NCHUNK = 12              # 12 chunks x 6 rows
CH_ROWS = 6
CLEN = CH_ROWS * ROW     # 444
INT_FLAT = H * W         # 5184

_DX = np.outer([1.0, 2.0, 1.0], [-1.0, 0.0, 1.0]).astype(np.float64) / 8.0
W1_EFF = _DX.T.astype(np.float32)   # sobel kernel 1 (applied depthwise)
W2_EFF = _DX.astype(np.float32)     # sobel kernel 2


# ---------------------------------------------------------------- threefry (bitwise jax-compatible)
def _rotl(x, d):
    return ((x << np.uint32(d)) | (x >> np.uint32(32 - d))).astype(np.uint32)


def _threefry2x32(ks0, ks1, x0, x1):
    ks0 = np.uint32(ks0); ks1 = np.uint32(ks1)
    ks2 = np.uint32(ks0 ^ ks1 ^ np.uint32(0x1BD11BDA))
    x0 = (x0 + ks0).astype(np.uint32); x1 = (x1 + ks1).astype(np.uint32)
    rot1 = [13, 15, 26, 6]; rot2 = [17, 29, 16, 24]
    ks = [ks1, ks2, ks0, ks1, ks2, ks0]
    for r in range(5):
        for rot in (rot1 if r % 2 == 0 else rot2):
            x0 = (x0 + x1).astype(np.uint32)
            x1 = _rotl(x1, rot)
            x1 = (x1 ^ x0).astype(np.uint32)
        x0 = (x0 + ks[r]).astype(np.uint32)
        x1 = (x1 + ks[r + 1] + np.uint32(r + 1)).astype(np.uint32)
    return x0, x1


def _fold_in(key, data):
    k0 = np.uint32(np.uint64(data) >> np.uint64(32))
    k1 = np.uint32(np.uint64(data) & np.uint64(0xFFFFFFFF))
    a, b = _threefry2x32(key[0], key[1], np.array([k0], np.uint32), np.array([k1], np.uint32))
    return np.uint32(a[0]), np.uint32(b[0])


def _uniform01(key, n):
    idx = np.arange(n, dtype=np.uint64)
    x0 = (idx >> np.uint64(32)).astype(np.uint32)
    x1 = (idx & np.uint64(0xFFFFFFFF)).astype(np.uint32)
    a, b = _threefry2x32(key[0], key[1], x0, x1)
    bits = a ^ b
    return (((bits >> np.uint32(9)) | np.uint32(0x3F800000)).view(np.float32)
            - np.float32(1.0))


def _all_uniforms():
    """u_n for n=0..T, shape (N_STEPS, B, H, W) fp32 — bitwise equal to
    jax.random.uniform(fold_in(key(42), n), (B,H,W))."""
    base = (np.uint32(0), np.uint32(42))
    out = np.empty((N_STEPS, B, H, W), np.float32)
    for n in range(N_STEPS):
        out[n] = _uniform01(_fold_in(base, n), B * H * W).reshape(B, H, W)
    return out


# ---------------------------------------------------------------- host-side packing
def _pad_map(img_chw):
    """[C,72,72] -> [C, MAPW] padded flat."""
    c = img_chw.shape[0]
    m = np.zeros((c, MAPW), np.float32)
    v = np.zeros((c, 74, 74), np.float32)
    v[:, 1:73, 1:73] = img_chw
    m[:, 2:2 + 74 * 74] = v.reshape(c, -1)
    return m


def _prep_weights(conv1_w, conv1_b, conv2_w, conv2_b, fc0_w, fc0_b, fc1_w):
    # dx-variant folded weight mats, lhsT layout [K, M]
    # perc channel order on device: [s1(0:16), s2(16:32), x(32:48)]
    perm = np.concatenate([np.arange(16, 32), np.arange(32, 48), np.arange(0, 16)])
    w1v = np.zeros((3, 48, 96), np.float32)   # [dy][perc_c, 32*vdx + o]  (slabs 32-spaced)
    w2v = np.zeros((3, 16, 96), np.float32)   # [dy][h1_c, 32*vdx + replica]
    wsv = np.zeros((3, 16, 96), np.float32)   # [dy][x_c, 32*vdx + (s1:c | s2:16+c)]
    for dy in range(3):
        for v in range(3):
            w1v[dy, :, 32 * v:32 * v + 16] = conv1_w[dy, v][perm]    # [48,16]
            w2v[dy, :, 32 * v:32 * v + 16] = np.repeat(conv2_w[dy, v], 16, axis=1)
            for cch in range(16):
                wsv[dy, cch, 32 * v + cch] = W1_EFF[dy, v]
                wsv[dy, cch, 32 * v + 16 + cch] = W2_EFF[dy, v]
    return dict(
        w1v=w1v, w2v=w2v, wsv=wsv,
        fc0t=fc0_w.astype(np.float32)[perm],      # [48,128]
        fc1t=fc1_w.astype(np.float32),            # [128,16]
        b0=fc0_b.reshape(128, 1).astype(np.float32),
        b1=conv1_b.reshape(16, 1).astype(np.float32),
        b2=np.full((16, 1), np.float32(conv2_b.reshape(-1)[0])),
        ones64=np.ones((1, 64), np.float32),
    )


# ---------------------------------------------------------------- device graph
ABLATE = set()


def _build_nc(n_steps):
    from contextlib import ExitStack
    import concourse.bass as bass
    import concourse.tile as tile
    from concourse import bacc, mybir

    DT = mybir.dt.float32
    DTR = mybir.dt.float32r
    AF = mybir.ActivationFunctionType
    OP = mybir.AluOpType
    NREC = n_steps - 1

    nc = bacc.Bacc("TRN2", target_bir_lowering=False, debug=False)

    xin = nc.declare_dram_parameter("xin", [16, MAPW], DT, isOutput=False)
    uin = nc.declare_dram_parameter("u", [n_steps, 16, INT_FLAT], DT, isOutput=False)
    wpar = {}
    for name, shape in [("w1v", [3, 48, 96]), ("w2v", [3, 16, 96]), ("wsv", [3, 16, 96]),
                        ("fc0t", [48, 128]), ("fc1t", [128, 16]),
                        ("b0", [128, 1]), ("b1", [16, 1]), ("b2", [16, 1]),
                        ("ones64", [1, 64])]:
        wpar[name] = nc.declare_dram_parameter(name, shape, DT, isOutput=False)

    xs_out = nc.declare_dram_parameter("xs", [NREC, 16, H, W], DT, isOutput=True)
    lam_out = nc.declare_dram_parameter("lam", [NREC, H, W], DT, isOutput=True)
    upd_out = nc.declare_dram_parameter("upd", [NREC, H, W], DT, isOutput=True)
    pn_out = nc.declare_dram_parameter("pn", [NREC, H, W], DT, isOutput=True)
    t_dram = nc.dram_tensor("t_bounce", [NREC, INT_FLAT], DT)

    with tile.TileContext(nc) as tc:
        with ExitStack() as ctx:
            wp = ctx.enter_context(tc.tile_pool(name="weights", bufs=1))
            st = ctx.enter_context(tc.tile_pool(name="state", bufs=1))
            sm = ctx.enter_context(tc.tile_pool(name="small", bufs=1))
            utp = ctx.enter_context(tc.tile_pool(name="upool", bufs=2))
            hp = ctx.enter_context(tc.tile_pool(name="hpool", bufs=2))
            tmp = ctx.enter_context(tc.tile_pool(name="tmp", bufs=4))
            pconv = ctx.enter_context(tc.tile_pool(name="pconv", bufs=4, space="PSUM"))
            pmlp = ctx.enter_context(tc.tile_pool(name="pmlp", bufs=2, space="PSUM"))
            pdx = ctx.enter_context(tc.tile_pool(name="pdx", bufs=2, space="PSUM"))

            # ---- persistent weights in SBUF (cast to fp32r via gpsimd DMA)
            w1v = [wp.tile([48, 96], DTR, tag=f"w1v{d}", name=f"w1v{d}") for d in range(3)]
            w2v = [wp.tile([16, 96], DTR, tag=f"w2v{d}", name=f"w2v{d}") for d in range(3)]
            wsv = [wp.tile([48, 96], DTR, tag=f"wsv{d}", name=f"wsv{d}") for d in range(3)]
            for d in range(3):
                nc.gpsimd.dma_start(w1v[d][:], wpar["w1v"][d])
                nc.gpsimd.dma_start(w2v[d][:], wpar["w2v"][d])
                nc.gpsimd.dma_start(wsv[d][32:48, :], wpar["wsv"][d])
            fc0t = wp.tile([48, 128], DTR, tag="fc0t", name="fc0t")
            fc1t = wp.tile([128, 16], DTR, tag="fc1t", name="fc1t")
            ones64 = wp.tile([1, 64], DTR, tag="ones64", name="ones64")
            ones64f = wp.tile([1, 64], DT, tag="ones64f", name="ones64f")
            nc.gpsimd.dma_start(fc0t[:], wpar["fc0t"][:])
            nc.gpsimd.dma_start(fc1t[:], wpar["fc1t"][:])
            nc.gpsimd.dma_start(ones64[:], wpar["ones64"][:])
            nc.sync.dma_start(ones64f[:], wpar["ones64"][:])
            b0 = wp.tile([128, 1], DT, tag="b0", name="b0")
            b1 = wp.tile([16, 1], DT, tag="b1", name="b1")
            b2 = wp.tile([16, 1], DT, tag="b2", name="b2")
            nc.sync.dma_start(b0[:], wpar["b0"][:])
            nc.sync.dma_start(b1[:], wpar["b1"][:])
            nc.sync.dma_start(b2[:], wpar["b2"][:])

            # ---- persistent state
            perc = [st.tile([48, MAPW], DTR, tag="percA", name="percA"),
                    st.tile([48, MAPW], DTR, tag="percB", name="percB")]
            xf = st.tile([16, MAPW], DT, tag="xf", name="xf")       # exact fp32 x
            h1 = st.tile([16, MAPW], DTR, tag="h1", name="h1")
            xn16 = st.tile([16, MAPW], DT, tag="xn16", name="xn16")
            lam16 = st.tile([16, MAPW], DT, tag="lam16", name="lam16")
            upd16 = st.tile([16, MAPW], DT, tag="upd16", name="upd16")
            un1 = st.tile([1, INT_FLAT], DT, tag="un1", name="un1")   # interior-flat un

            a72 = [sm.tile([72, 74], DT, tag=f"a72_{i}", name=f"a72_{i}") for i in range(2)]
            mp = [sm.tile([72, 74], DT, tag=f"mp{i}", name=f"mp{i}") for i in range(4)]
            pre72 = sm.tile([72, 74], DT, tag="pre72", name="pre72")
            lif72 = sm.tile([72, 74], DT, tag="lif72", name="lif72")

            for t_ in (perc[0], perc[1], xf, h1,
                       a72[0], a72[1], mp[0], mp[1], mp[2], mp[3]):
                ap = t_[:]
                if ap.dtype == DTR:
                    ap = ap.bitcast(DT)
                nc.gpsimd.memset(ap, 0.0)
            nc.gpsimd.memset(un1[:], 1.0)

            # 3-D views (74 rows x 74 cols)
            def v3(t, p0, p1):
                return t[p0:p1, 2:2 + 74 * 74].rearrange("p (h w) -> p h w", h=74)

            xf3 = v3(xf, 0, 16)
            xn3 = v3(xn16, 0, 16)
            lam3 = v3(lam16, 0, 16)
            upd3 = v3(upd16, 0, 16)
            unF3 = un1[0:1, :].rearrange("p (h w) -> p h w", h=72)
            # t = -un*lam scratch lives in upd16 row 0 (holds prev-step upd,
            # already DMA'd out by the time t is written)
            tF3 = v3(upd16, 0, 1)

            # load x into xf (fp32) and perc[1] x-block (fp32r)
            nc.sync.dma_start(xf[:], xin[:])
            nc.gpsimd.dma_start(perc[1][32:48, :], xin[:])

            def chunk_cols(j):
                return 2 + ROW + CLEN * j          # start col of chunk j (6 rows incl pads)

            def conv3(dst_psum, m, src, src_p0, src_k, wts, j, extend=1):
                """3 dy-matmuls accumulating into psum [m, CLEN+2*extend]."""
                c0 = chunk_cols(j) - extend
                ln = CLEN + 2 * extend
                for di, dy in enumerate((-1, 0, 1)):
                    s = c0 + dy * ROW
                    nc.tensor.matmul(dst_psum[0:m, 0:ln],
                                     wts[di][src_p0:src_p0 + src_k, :],
                                     src[src_p0:src_p0 + src_k, s:s + ln],
                                     start=(di == 0), stop=(di == 2))

            def combine3(psum, nslab, dst3, dst_p0, rows_j, bias_ap, act, dtag,
                         eng_b=None):
                """dst3[:, r0:r0+6, 1:73] = act(V-1[c-1]+V0[c]+V1[c+1]+bias).
                One PSUM operand per instruction: evacuate slab0 via ACT."""
                w_ = nslab
                engB = eng_b if eng_b is not None else nc.vector
                tA = tmp.tile([w_, CLEN + 2], DT, tag="tmp", name=f"cmbA{dtag}")
                if bias_ap is not None:
                    nc.scalar.activation(tA[:, 0:CLEN], psum[0:w_, 0:CLEN],
                                         AF.Identity, bias=bias_ap)
                else:
                    nc.scalar.activation(tA[:, 0:CLEN], psum[0:w_, 0:CLEN],
                                         AF.Identity)
                tB = tmp.tile([w_, CLEN + 2], DT, tag="tmp", name=f"cmbB{dtag}")
                nc.vector.tensor_tensor(
                    tB[:, 0:CLEN], tA[:, 0:CLEN],
                    psum[32:32 + w_, 1:1 + CLEN], OP.add)
                r0 = 1 + CH_ROWS * rows_j
                dst = dst3[dst_p0:dst_p0 + w_, r0:r0 + CH_ROWS, 1:73]
                if act is None:
                    nc.vector.tensor_tensor(
                        tA[:, 0:CLEN], tB[:, 0:CLEN],
                        psum[64:64 + w_, 2:2 + CLEN], OP.add)
                    tCv = tA[:, 0:CLEN].rearrange("p (h w) -> p h w", h=CH_ROWS)
                    engB.tensor_copy(dst, tCv[:, :, 1:73])
                else:
                    tC = tmp.tile([w_, CLEN + 2], DT, tag="tmp", name=f"cmbC{dtag}")
                    nc.vector.tensor_tensor(
                        tC[:, 0:CLEN], tB[:, 0:CLEN],
                        psum[64:64 + w_, 2:2 + CLEN], OP.add)
                    tCv = tC[:, 0:CLEN].rearrange("p (h w) -> p h w", h=CH_ROWS)
                    nc.scalar.activation(dst, tCv[:, :, 1:73], act)

            def maxpool_gt(src72, out72, w1t, w2t, shu, shd):
                """out72 = (3x3 maxpool(src) > ALIVE_TH); DMA row shifts for the
                h direction (engine partition bases must be 32-aligned)."""
                nc.vector.tensor_tensor(w1t[:, 1:73], src72[:, 0:72], src72[:, 2:74], OP.max)
                nc.vector.tensor_tensor(w2t[:, 1:73], w1t[:, 1:73], src72[:, 1:73], OP.max)
                nc.sync.dma_start(shu[0:71, 1:73], w2t[1:72, 1:73])
                nc.sync.dma_start(shd[1:72, 1:73], w2t[0:71, 1:73])
                nc.vector.tensor_tensor(w1t[:, 1:73], shu[:, 1:73], shd[:, 1:73], OP.max)
                nc.vector.tensor_tensor(w1t[:, 1:73], w1t[:, 1:73], w2t[:, 1:73], OP.max)
                nc.vector.tensor_scalar(out72[:, 1:73], w1t[:, 1:73], ALIVE_TH, None, OP.is_gt)

            # ---------------- one full step ----------------
            def body(n):
                rec = n >= 1
                cur = perc[1] if n <= 1 else (perc[0] if n % 2 == 0 else perc[1])
                nxt = perc[1] if n == 0 else (perc[0] if n % 2 == 1 else perc[1])
                xdst = perc[0] if n % 2 == 0 else perc[1]
                # note: cur=home(X_{n-1}), nxt=home(X_n), xdst=home(X_{n+1})

                # --- record prev-step upd + this step's x BEFORE overwrites
                if rec and "outs" not in ABLATE:
                    nc.sync.dma_start(upd_out[n - 1], upd3[0:1, 1:73, 1:73])
                    nc.sync.dma_start(xs_out[n - 1], xf3[:, 1:73, 1:73])


                # --- pre-alive from current x
                if "prealive" not in ABLATE:
                    nc.sync.dma_start(a72[0][:, 1:73], xf3[3:4, 1:73, 1:73])
                    maxpool_gt(a72[0], pre72, mp[0], mp[1], mp[2], mp[3])

                # --- lambda path: conv1 on stale perc -> h1 -> conv2 -> lam
                for j in range(NCHUNK):
                    pc = pconv.tile([96, CLEN + 2], DT, tag="pconv", name="pconv")
                    conv3(pc, 96, cur, 0, 48, w1v, j)
                    combine3(pc, 16, v3(h1, 0, 16), 0, j, b1[:, 0:1], AF.Relu, "h1", eng_b=nc.gpsimd)
                for j in range(NCHUNK):
                    pc = pconv.tile([96, CLEN + 2], DT, tag="pconv", name="pconv")
                    conv3(pc, 96, h1, 0, 16, w2v, j)
                    combine3(pc, 16, lam3, 0, j, b2[:, 0:1], AF.Sigmoid, "lam")

                # --- un / t / upd (pointwise)
                if rec and "unt" not in ABLATE:
                    nc.vector.scalar_tensor_tensor(
                        tF3[:, 1:73, 1:73], lam3[0:1, 1:73, 1:73], -1.0,
                        unF3[:, :, :], OP.mult, OP.mult)        # t = -lam*un
                    nc.vector.tensor_tensor(
                        unF3[:, :, :], unF3[:, :, :], tF3[:, 1:73, 1:73], OP.add)  # un += t
                    nc.sync.dma_start(lam_out[n - 1], lam3[0:1, 1:73, 1:73])
                    nc.sync.dma_start(t_dram[n - 1], tF3[0:1, 1:73, 1:73])
                for g in range(4 if "upd" not in ABLATE else 0):
                    uc = utp.tile([16, 1296], DT, tag="uc", name="uc")
                    nc.sync.dma_start(uc[:], uin[n, :, 1296 * g:1296 * (g + 1)])
                    r0 = 1 + 18 * g
                    uc3 = uc[:, :].rearrange("p (h w) -> p h w", h=18)
                    nc.vector.tensor_tensor(
                        upd3[:, r0:r0 + 18, 1:73], uc3[:, :, :],
                        lam3[:, r0:r0 + 18, 1:73], OP.is_ge)

                # --- sobel: perceive X_n s-blocks into nxt
                if n >= 1 and "sobel" not in ABLATE:
                    for j in range(NCHUNK):
                        ps = pconv.tile([96, CLEN + 2], DT, tag="pconv", name="pconv")
                        conv3(ps, 96, nxt, 32, 16, wsv, j)
                        combine3(ps, 32, v3(nxt, 0, 32), 0, j, None, None, "sob", eng_b=nc.gpsimd)

                # --- MLP + x update per chunk
                for j in range(NCHUNK if "mlp" not in ABLATE else 0):
                    c0 = chunk_cols(j)
                    ph = pmlp.tile([128, CLEN], DT, tag="ph", name="ph")
                    nc.tensor.matmul(ph[:], fc0t[:], nxt[0:48, c0:c0 + CLEN],
                                     start=True, stop=True)
                    ht = hp.tile([128, CLEN], DTR, tag="ht", name="ht", bufs=3)
                    nc.scalar.activation(ht[:], ph[:], AF.Relu, bias=b0[:, 0:1])
                    pd = pdx.tile([16, CLEN], DT, tag="pd", name="pd")
                    nc.tensor.matmul(pd[:], fc1t[:], ht[:], start=True, stop=True)
                    # xn = x + dx*upd   (interior rows of this chunk)
                    r0 = 1 + CH_ROWS * j
                    pd3 = pd[:, :].rearrange("p (h w) -> p h w", h=CH_ROWS)
                    tdx = tmp.tile([16, CLEN], DT, tag="tmp", name="tdx")
                    tdx3 = tdx[:, :].rearrange("p (h w) -> p h w", h=CH_ROWS)
                    nc.vector.tensor_tensor(
                        tdx3[:, :, 1:73], pd3[:, :, 1:73],
                        upd3[:, r0:r0 + CH_ROWS, 1:73], OP.mult)
                    nc.vector.tensor_tensor(
                        xn3[:, r0:r0 + CH_ROWS, 1:73], tdx3[:, :, 1:73],
                        xf3[:, r0:r0 + CH_ROWS, 1:73], OP.add)

                # --- life = pre & alive(xn)
                if "life" in ABLATE:
                    return
                nc.sync.dma_start(a72[1][:, 1:73], xn3[3:4, 1:73, 1:73])
                maxpool_gt(a72[1], lif72, mp[0], mp[1], mp[2], mp[3])
                nc.vector.tensor_tensor(lif72[:, 1:73], lif72[:, 1:73],
                                        pre72[:, 1:73], OP.mult)
                # broadcast life -> 16 partitions via K=1 matmul, then
                # multiply xn by life straight out of PSUM per chunk
                ones16 = ones64[0:1, 0:16]
                xd3 = v3(xdst, 32, 48)
                for j in range(NCHUNK):
                    lifc = utp.tile([1, 432], DTR, tag="lifc", name="lifc")
                    nc.gpsimd.dma_start(lifc[:], lif72[6 * j:6 * j + 6, 1:73])
                    pb = pdx.tile([16, CLEN], DT, tag="pd", name="pd")
                    nc.tensor.matmul(pb[0:16, 0:432], ones16, lifc[:],
                                     start=True, stop=True)
                    r0 = 1 + CH_ROWS * j
                    pb3 = pb[:, 0:432].rearrange("p (h w) -> p h w", h=CH_ROWS)
                    nc.vector.tensor_tensor(xf3[:, r0:r0 + CH_ROWS, 1:73],
                                            xn3[:, r0:r0 + CH_ROWS, 1:73],
                                            pb3[:, :, :], OP.mult)
                    nc.gpsimd.tensor_copy(xd3[:, r0:r0 + CH_ROWS, 1:73],
                                           xf3[:, r0:r0 + CH_ROWS, 1:73])

            # initial sobel of x_orig into perc[1] (body n=0 skips sobel)
            for j in range(NCHUNK):
                ps = pconv.tile([96, CLEN + 2], DT, tag="pconv", name="pconv")
                conv3(ps, 96, perc[1], 32, 16, wsv, j)
                combine3(ps, 32, v3(perc[1], 0, 32), 0, j, None, None, "sob")

            for n in range(n_steps):
                body(n)

            # ---------------- p normalization ----------------
            # t_n = -q_n;  P_n = (t_n - eps) * recip(sum_n t_n - 64*eps)
            import bass_rust
            ts = st.tile([64, INT_FLAT], DT, tag="lam16", name="ts")
            if NREC < 64:
                nc.gpsimd.memset(ts[:], 0.0)
            nc.sync.dma_start(ts[0:NREC, :], t_dram[:, :])
            sr64 = st.tile([64, INT_FLAT], DT, tag="upd16", name="sr64")
            nc.gpsimd.partition_all_reduce(sr64[:], ts[:], 64,
                                           bass_rust.ReduceOp.add)
            nc.vector.tensor_scalar(sr64[:], sr64[:], -float(NREC) * P_EPS,
                                    None, OP.add)
            r64 = st.tile([64, INT_FLAT], DT, tag="xn16", name="r64")
            nc.vector.reciprocal(r64[:], sr64[:])
            po = st.tile([64, INT_FLAT], DT, tag="xf", name="po")
            nc.vector.scalar_tensor_tensor(po[:], ts[:], P_EPS, r64[:],
                                           OP.subtract, OP.mult)
            nc.sync.dma_start(pn_out[:, :, :].rearrange("n h w -> n (h w)"),
                              po[0:NREC, :])
    nc.compile()
    return nc


_NC_CACHE = {}


def _get_nc(n_steps):
    if n_steps not in _NC_CACHE:
        _NC_CACHE[n_steps] = _build_nc(n_steps)
    return _NC_CACHE[n_steps]


# ---------------------------------------------------------------- entry point
def kernel(x, conv1_w, conv1_b, conv2_w, conv2_b, fc0_w, fc0_b, fc1_w,
           n_steps=N_STEPS, _return_raw=False, _trace=False):
    from concourse.bass_utils import run_bass_kernel_spmd

    x = np.asarray(x, np.float32)
    wd = _prep_weights(np.asarray(conv1_w), np.asarray(conv1_b),
                       np.asarray(conv2_w), np.asarray(conv2_b),
                       np.asarray(fc0_w), np.asarray(fc0_b), np.asarray(fc1_w))
    u_all = _all_uniforms()[:n_steps]          # (n_steps, B, H, W)

    nc = _get_nc(n_steps)
    in_maps = []
    for b in range(B):
        m = dict(wd)
        m["xin"] = _pad_map(x[b].transpose(2, 0, 1))
        uflat = u_all[:n_steps, b].reshape(n_steps, 1, INT_FLAT)
        m["u"] = np.repeat(uflat, 16, axis=1).astype(np.float32)
        in_maps.append(m)

    res = run_bass_kernel_spmd(nc, in_maps, core_ids=list(range(B)), trace=_trace)
    nrec = n_steps - 1
    x_steps = np.zeros((nrec, B, H, W, C), np.float32)
    p_norm = np.zeros((nrec, B, H, W), np.float32)
    lam_steps = np.zeros((nrec, B, H, W), np.float32)
    upd_steps = np.zeros((nrec, B, H, W), np.float32)
    for b in range(B):
        r = res.results[b]
        x_steps[:, b] = r["xs"].transpose(0, 2, 3, 1)
        p_norm[:, b] = r["pn"]
        lam_steps[:, b] = r["lam"]
        upd_steps[:, b] = r["upd"]
    if _return_raw:
        return (x_steps, p_norm, lam_steps, upd_steps), res
    return x_steps, p_norm, lam_steps, upd_steps
